# revision 2
# baseline (speedup 1.0000x reference)
"""SRU stack (5 layers + FC head) on Trainium2, batch-sharded across 8 NeuronCores.

Model (per sample):
    for each layer l:  U = W_l @ h          (h: [H, t] transposed layout)
                       f = sigmoid(zf + bf); r = sigmoid(zr + br)
                       c_t = f_t * c_{t-1} + (1 - f_t) * xt_t      (time scan)
                       h   = r * c + (1 - r) * h_in                (highway)
    out = fc_W @ h + fc_b

Kernel layout choices:
  * Everything on-chip lives transposed: [feature (SBUF partition), (batch, time) (free)].
    The host pre-transposes x / Ws / fc_W, so no on-chip transposes are needed.
  * Matmul operands are fp16 (full PE rate, ~1e-3 quantization); accumulation,
    gates and the scan are fp32.
  * The time recurrence uses the DVE's native tensor_tensor_scan:
        state = (data0 * state) op1 data1   along the free dim, fp32 state.
    With gneg = (f - 1) * xt (one fused scalar_tensor_tensor op) the SRU cell is
        c = scan(f, gneg, op0=mult, op1=subtract)  ->  c = f*c_prev + (1-f)*xt.
  * Highway uses h = c + (r - 1) * (c - h_in):
        d = c - h_in            (GPSIMD)
        d = (r - 1) * d         (DVE fused scalar_tensor_tensor, in place)
        h = c + d -> fp16       (GPSIMD)
"""

from contextlib import ExitStack

import numpy as np

import concourse.bass as bass
import concourse.bacc as bacc
import concourse.mybir as mybir
import concourse.tile as tile
from concourse.bass_utils import run_bass_kernel_spmd

SEQ, BATCH, HID, OUT, NLAYERS = 2048, 16, 512, 10, 5
NCORES = 8
BC = BATCH // NCORES       # batch per core = 2
HC = HID // 128            # hidden 128-chunks = 4
T = 128                    # time-chunk

F32 = mybir.dt.float32
F16 = mybir.dt.float16
Sigmoid = mybir.ActivationFunctionType.Sigmoid
Alu = mybir.AluOpType


def build(seq=SEQ):
    """Build the single-core Bass module (SPMD: same NEFF on all 8 cores)."""
    nch = seq // T
    nc = bacc.Bacc("TRN2", target_bir_lowering=False, debug=False)
    xT = nc.dram_tensor("xT", [HID, BC, seq], F16, kind="ExternalInput").ap()
    WT = nc.dram_tensor("WT", [NLAYERS, HID, 3 * HID], F16, kind="ExternalInput").ap()
    bT = nc.dram_tensor("bT", [128, NLAYERS, 2, HC], F32, kind="ExternalInput").ap()
    fWT = nc.dram_tensor("fWT", [HID, OUT], F16, kind="ExternalInput").ap()
    fb = nc.dram_tensor("fb", [OUT, 1], F32, kind="ExternalInput").ap()
    outT = nc.dram_tensor("outT", [OUT, BC, seq], F32, kind="ExternalOutput").ap()

    with tile.TileContext(nc) as tc, ExitStack() as ctx:
        wpool = ctx.enter_context(tc.tile_pool(name="w", bufs=1))
        hpool = ctx.enter_context(tc.tile_pool(name="h", bufs=2))
        fpool = ctx.enter_context(tc.tile_pool(name="fp", bufs=2))
        rpool = ctx.enter_context(tc.tile_pool(name="rp", bufs=2))
        gpool = ctx.enter_context(tc.tile_pool(name="gp", bufs=2))
        cpool = ctx.enter_context(tc.tile_pool(name="cp", bufs=3))
        dpool = ctx.enter_context(tc.tile_pool(name="dp", bufs=2))
        opool = ctx.enter_context(tc.tile_pool(name="op", bufs=2))
        psum = ctx.enter_context(tc.tile_pool(name="ps", bufs=6, space="PSUM"))
        fcps = ctx.enter_context(tc.tile_pool(name="fcps", bufs=2, space="PSUM"))
        cons = ctx.enter_context(tc.tile_pool(name="cons", bufs=1))

        # ---- resident weights / constants ----
        w_sb = []
        for l in range(NLAYERS):
            per = []
            for kc in range(HC):
                wt = wpool.tile([128, 3 * HID], F16, name=f"w{l}_{kc}", tag=f"w{l}_{kc}")
                nc.sync.dma_start(wt[:], WT[l, kc * 128:(kc + 1) * 128, :])
                per.append(wt)
            w_sb.append(per)
        bias = cons.tile([128, NLAYERS, 2, HC], F32, name="bias", tag="bias")
        nc.sync.dma_start(bias[:], bT[:])
        fw = cons.tile([128, HC, OUT], F16, name="fw", tag="fw")
        for kc in range(HC):
            nc.sync.dma_start(fw[:, kc], fWT[kc * 128:(kc + 1) * 128, :])
        fbt = cons.tile([OUT, 1], F32, name="fbt", tag="fbt")
        nc.sync.dma_start(fbt[:], fb[:])

        # ---- input activations (fp16, transposed) ----
        hcur = hpool.tile([128, HC, BC, seq], F16, name="hbuf", tag="hbuf")
        for kc in range(HC):
            nc.sync.dma_start(hcur[:, kc], xT[kc * 128:(kc + 1) * 128])

        # ---- SRU layers (layer-major; scan chains chunks via `initial`) ----
        for l in range(NLAYERS):
            hnext = hpool.tile([128, HC, BC, seq], F16, name="hbuf", tag="hbuf")
            c_prev = None
            for k in range(nch):
                ts = slice(k * T, (k + 1) * T)
                f_t = fpool.tile([128, HC, BC, T], F32, name="f_t", tag="f_t")
                r_t = rpool.tile([128, HC, BC, T], F32, name="r_t", tag="r_t")
                g_t = gpool.tile([128, HC, BC, T], F32, name="g_t", tag="g_t")
                c_t = cpool.tile([128, HC, BC, T], F32, name="c_t", tag="c_t")
                d_t = dpool.tile([128, HC, BC, T], F32, name="d_t", tag="d_t")
                # zf rows first (f gate), then zr, then xt (consumed with f).
                for mc in list(range(HC, 2 * HC)) + list(range(2 * HC, 3 * HC)) + list(range(HC)):
                    ps = psum.tile([128, BC, T], F32, name="ups", tag="ups")
                    for kc in range(HC):
                        nc.tensor.matmul(
                            ps[:],
                            lhsT=w_sb[l][kc][:, mc * 128:(mc + 1) * 128],
                            rhs=hcur[:, kc, :, ts],
                            start=(kc == 0),
                            stop=(kc == HC - 1),
                        )
                    hco = mc % HC
                    if mc < HC:
                        # gneg = (f - 1) * xt
                        nc.vector.scalar_tensor_tensor(
                            out=g_t[:, hco], in0=f_t[:, hco], scalar=1.0, in1=ps[:],
                            op0=Alu.subtract, op1=Alu.mult)
                    elif mc < 2 * HC:
                        nc.scalar.activation(f_t[:, hco], ps[:], Sigmoid,
                                             bias=bias[:, l, 0, hco:hco + 1], scale=1.0)
                    else:
                        nc.scalar.activation(r_t[:, hco], ps[:], Sigmoid,
                                             bias=bias[:, l, 1, hco:hco + 1], scale=1.0)
                # c = f * c_prev + (1 - f) * xt  == scan(f, gneg; mult, subtract)
                for hci in range(HC):
                    for b in range(BC):
                        init = 0.0 if k == 0 else c_prev[:, hci, b, T - 1:T]
                        nc.vector.tensor_tensor_scan(
                            out=c_t[:, hci, b], data0=f_t[:, hci, b],
                            data1=g_t[:, hci, b], initial=init,
                            op0=Alu.mult, op1=Alu.subtract)
                # h = c + (r - 1) * (c - h_in)
                nc.gpsimd.tensor_sub(d_t[:], c_t[:], hcur[:, :, :, ts])
                nc.vector.scalar_tensor_tensor(
                    out=d_t[:], in0=r_t[:], scalar=1.0, in1=d_t[:],
                    op0=Alu.subtract, op1=Alu.mult)
                nc.gpsimd.tensor_add(hnext[:, :, :, ts], c_t[:], d_t[:])
                c_prev = c_t
            hcur = hnext

        # ---- FC head ----
        for k in range(nch):
            ts = slice(k * T, (k + 1) * T)
            ps = fcps.tile([OUT, BC, T], F32, name="fps", tag="fps")
            for kc in range(HC):
                nc.tensor.matmul(ps[:], lhsT=fw[:, kc], rhs=hcur[:, kc, :, ts],
                                 start=(kc == 0), stop=(kc == HC - 1))
            o_t = opool.tile([OUT, BC, T], F32, name="o_t", tag="o_t")
            nc.vector.tensor_scalar_add(o_t[:], ps[:], fbt[:])
            nc.sync.dma_start(outT[:, :, ts], o_t[:])
    nc.compile()
    return nc


def prep_inputs(x, Ws, bs, fc_W, fc_b):
    """Host-side reshape/cast into the kernel's transposed fp16 layouts."""
    x = np.asarray(x, np.float32)
    xT = np.ascontiguousarray(x.transpose(2, 1, 0)).astype(np.float16)  # [H, B, L]
    WT = np.ascontiguousarray(
        np.asarray(Ws, np.float32).transpose(0, 2, 1)).astype(np.float16)
    bT = np.ascontiguousarray(
        np.asarray(bs, np.float32).reshape(NLAYERS, 2, HC, 128).transpose(3, 0, 1, 2))
    fWT = np.ascontiguousarray(np.asarray(fc_W, np.float32).T).astype(np.float16)
    fb = np.asarray(fc_b, np.float32).reshape(OUT, 1)
    in_maps = []
    for c in range(NCORES):
        xc = np.ascontiguousarray(xT[:, c * BC:(c + 1) * BC, :])
        in_maps.append({"xT": xc, "WT": WT, "bT": bT, "fWT": fWT, "fb": fb})
    return in_maps


_BUILT = {}


def get_built(seq=SEQ):
    if seq not in _BUILT:
        _BUILT[seq] = build(seq)
    return _BUILT[seq]


def run(inputs, trace=False):
    """Run on the 8 NeuronCores; returns (full output, BassKernelResults)."""
    nc = get_built()
    in_maps = prep_inputs(**inputs)
    res = run_bass_kernel_spmd(nc, in_maps, core_ids=list(range(NCORES)), trace=trace)
    out = np.empty((SEQ, BATCH, OUT), np.float32)
    for c in range(NCORES):
        out[:, c * BC:(c + 1) * BC, :] = res.results[c]["outT"].transpose(2, 1, 0)
    return out, res


def kernel(**inputs) -> np.ndarray:
    out, _ = run(inputs)
    return out
